# revision 3
# baseline (speedup 1.0000x reference)
"""Trainium2 Bass kernel for DrugProteinModel (SAGEConv GNN + pooling + MLP head).

Strategy (8 NeuronCores, fully graph-parallel, zero collectives):
  - batch is sorted, so graphs [c*128,(c+1)*128) map to a contiguous node-id
    range per core; edges are assigned to the core owning their dst node.
  - Each core gathers x[src] rows straight from HBM with dma_gather (int16
    indices -> two src buckets with a zero pad row each), aggregates them per
    dst via on-chip one-hot matrices multiplied on the TensorEngine
    (aggrT[64, dst] accumulated in PSUM), applies 1/deg, runs the SAGE linear
    + ReLU, pools per graph with a second one-hot matmul, and finishes the
    protein/interaction/output head entirely on-core for its own 128 graphs.
  - Host-side numpy does sharding/layout only (sort edges by dst, build idx
    tables, pack weights); all per-edge math runs on the NeuronCores.
"""
import sys

sys.path.insert(0, "/opt/trn_rl_repo")

import numpy as np
import ml_dtypes

import concourse.bacc as bacc
import concourse.mybir as mybir
from concourse.tile import TileContext
from concourse import library_config
from concourse.bass_utils import run_bass_kernel_spmd

dt = mybir.dt
f32 = np.float32
bf16 = ml_dtypes.bfloat16

NCORES = 8
D = 64      # node feature dim
HID = 256
DP = 1280   # protein dim
GB = 6      # blocks per gather instruction group


def _wrap_idxs(flat):
    """[n] int16 -> [128, n//16]: idx i at [i%16, i//16], replicated for 8 Q7 cores."""
    n = len(flat)
    assert n % 16 == 0
    w = flat.reshape(n // 16, 16).T
    return np.tile(w, (8, 1)).astype(np.int16)


def preprocess(x, protein_embedding, Wl, bl, Wr, Wp, bp, Wi, bi, Wo, bo,
               edge_index, batch, ncores=NCORES, split=None):
    N, d = x.shape
    E = edge_index.shape[1]
    G = protein_embedding.shape[0]
    GPC = G // ncores
    assert d == D

    src = edge_index[0].astype(np.int64)
    dst = edge_index[1].astype(np.int64)
    batch = np.asarray(batch).astype(np.int64)

    SPLIT = split if split is not None else min(32767, N)  # src < SPLIT -> bucket A

    # ---- shard nodes by graph range (batch sorted -> contiguous node ids)
    gb_bounds = np.searchsorted(batch, np.arange(0, G + 1, GPC))  # [ncores+1]

    deg = np.bincount(dst, minlength=N)
    a_cnt = np.bincount(dst[src < SPLIT], minlength=N)
    b_cnt = deg - a_cnt

    # ---- edges sorted by (dst, bucket)
    key = dst * 2 + (src >= SPLIT)
    order = np.argsort(key, kind="stable")
    src_s, dst_s = src[order], dst[order]

    # ---- choose per-block chunk budgets (CPA, CPB2) and pack nodes into blocks
    def pack_core(c, CPA, CPB2):
        n0, n1 = gb_bounds[c], gb_bounds[c + 1]
        ca = np.concatenate([[0], np.cumsum(a_cnt[n0:n1])])
        cb = np.concatenate([[0], np.cumsum(b_cnt[n0:n1])])
        nloc = n1 - n0
        blocks = []
        s = 0
        while s < nloc:
            e1 = np.searchsorted(ca, ca[s] + CPA * 128, "right") - 1
            e2 = np.searchsorted(cb, cb[s] + CPB2 * 128, "right") - 1
            e = min(e1, e2, s + 128, nloc)
            assert e > s
            blocks.append((n0 + s, n0 + e))
            s = e
        return blocks

    mean_deg = E / max(N, 1)
    ca0 = max(1, int(np.ceil(128 * mean_deg * (SPLIT / N) / 128)))
    cb0 = max(1, int(np.ceil(128 * mean_deg * (1 - SPLIT / N) / 128))) if SPLIT < N else 1
    best = None
    for CPA in range(max(1, ca0 - 1), ca0 + 3):
        for CPB2 in range(max(1, cb0 - 1), cb0 + 3):
            nb = max(len(pack_core(c, CPA, CPB2)) for c in range(ncores))
            cost = nb * (CPA + CPB2)
            if best is None or cost < best[0]:
                best = (cost, CPA, CPB2, nb)
    _, CPA, CPB2, NB = best
    CPB_TOT = CPA + CPB2
    NODES_PC = NB * 128
    CH = NB * CPB_TOT
    NGRP = (NB + GB - 1) // GB

    # ---- gather table: [0, x[0:SPLIT], 0, x[SPLIT:]] rows padded to 128 cols, bf16
    TAB = N + 2
    tab = np.zeros((TAB, 128), f32)
    tab[1:1 + SPLIT, :D] = x[:SPLIT]
    tab[SPLIT + 2:SPLIT + 2 + (N - SPLIT), :D] = x[SPLIT:]
    tab_bf16 = tab.astype(bf16)
    B1 = SPLIT + 1  # base row of bucket B (its zero row)

    invdeg_full = (1.0 / np.maximum(deg, 1)).astype(f32)
    cnt_g = np.bincount(batch, minlength=G)
    invcnt_full = (1.0 / np.maximum(cnt_g, 1)).astype(f32)

    # ---- shared constants
    iota = np.tile(np.arange(128, dtype=f32)[None, :], (128, 1))
    ident = np.eye(128, dtype=f32)
    onesrow = np.zeros((128, 128), f32)
    onesrow[0, :] = 1.0

    # ---- packed weights (shared across cores)
    Wl_b = np.asarray(Wl).astype(bf16)                                   # [64,256]
    Wr_bl = np.concatenate([np.asarray(Wr), np.asarray(bl)[None, :]], 0).astype(bf16)  # [65,256]

    KP = (DP + 1 + 127) // 128  # protein K chunks (incl bias row)
    Wp_rows = np.zeros((KP * 128, HID), f32)
    Wp_rows[:DP] = Wp
    Wp_rows[DP] = bp
    Wp_pack = np.zeros((128, KP * HID), f32)
    for k in range(KP):
        Wp_pack[:, k * HID:(k + 1) * HID] = Wp_rows[k * 128:(k + 1) * 128]

    KI = (2 * HID) // 128  # 4
    Wi_pack = np.zeros((128, KI * HID), f32)
    for k in range(KI):
        Wi_pack[:, k * HID:(k + 1) * HID] = Wi[k * 128:(k + 1) * 128]
    bi_pack = np.zeros((128, HID), f32)
    bi_pack[0] = bi

    KO = HID // 128  # 2
    Wo_pack = np.zeros((128, KO), f32)
    for k in range(KO):
        Wo_pack[:, k] = Wo[k * 128:(k + 1) * 128, 0]
    bo_pack = np.zeros((128, 1), f32)
    bo_pack[0, 0] = bo[0]

    dims = dict(N=N, E=E, G=G, GPC=GPC, ncores=ncores, SPLIT=SPLIT, TAB=TAB,
                NB=NB, CPA=CPA, CPB2=CPB2, NODES_PC=NODES_PC, CH=CH,
                NGRP=NGRP, KP=KP, KI=KI, KO=KO,
                LA16=NB * CPA * 8, LB16=NB * CPB2 * 8)

    # ---- per-core tables
    in_maps = []
    for c in range(ncores):
        blocks = pack_core(c, CPA, CPB2)
        nb_c = len(blocks)

        idxA = np.zeros((NB, CPA * 128), np.int16)
        idxB = np.zeros((NB, CPB2 * 128), np.int16)
        dstloc = np.zeros((128, CH), f32)
        invdeg_bc = np.zeros((64, NODES_PC), f32)
        batch_local = np.full((128, NB), -1.0, f32)
        node_of_slot = np.full(NODES_PC, -1, np.int64)

        for b, (ns, ne) in enumerate(blocks):
            nn = ne - ns
            node_of_slot[b * 128:b * 128 + nn] = np.arange(ns, ne)
            e0 = np.searchsorted(dst_s, ns, "left")
            e1 = np.searchsorted(dst_s, ne, "left")
            es, ds = src_s[e0:e1], dst_s[e0:e1] - ns
            mA = es < SPLIT
            iA, dA = (es[mA] + 1).astype(np.int16), ds[mA].astype(f32)
            iB, dB = (es[~mA] - SPLIT + 1).astype(np.int16), ds[~mA].astype(f32)
            assert len(iA) <= CPA * 128 and len(iB) <= CPB2 * 128
            idxA[b, :len(iA)] = iA
            idxB[b, :len(iB)] = iB
            col0 = b * CPB_TOT
            dla = np.zeros(CPA * 128, f32)
            dla[:len(dA)] = dA
            dstloc[:, col0:col0 + CPA] = dla.reshape(CPA, 128).T
            dlb = np.zeros(CPB2 * 128, f32)
            dlb[:len(dB)] = dB
            dstloc[:, col0 + CPA:col0 + CPB_TOT] = dlb.reshape(CPB2, 128).T
            invdeg_bc[:, b * 128:b * 128 + nn] = invdeg_full[ns:ne][None, :]
            batch_local[:nn, b] = (batch[ns:ne] - c * GPC).astype(f32)

        # wrap idx streams per gather group
        wA = [_wrap_idxs(idxA[g * GB:min(NB, (g + 1) * GB)].reshape(-1))
              for g in range(NGRP)]
        wB = [_wrap_idxs(idxB[g * GB:min(NB, (g + 1) * GB)].reshape(-1))
              for g in range(NGRP)]
        idxA_w = np.concatenate(wA, axis=1)
        idxB_w = np.concatenate(wB, axis=1)

        # xT with ones row, in slot order
        xT = np.zeros((65, NODES_PC), f32)
        real = node_of_slot >= 0
        xT[:D, real] = np.asarray(x)[node_of_slot[real]].T
        xT[D, :] = 1.0
        xT_bf = xT.astype(bf16)

        invcnt = np.zeros((128, 1), f32)
        invcnt[:GPC, 0] = invcnt_full[c * GPC:(c + 1) * GPC]

        peT_pack = np.zeros((128, KP * 128), f32)
        pe_rows = np.zeros((KP * 128, 128), f32)
        pe_rows[:DP, :GPC] = np.asarray(protein_embedding)[c * GPC:(c + 1) * GPC].T
        pe_rows[DP, :] = 1.0
        for k in range(KP):
            peT_pack[:, k * 128:(k + 1) * 128] = pe_rows[k * 128:(k + 1) * 128]

        in_maps.append(dict(
            tab=tab_bf16, idxA=idxA_w, idxB=idxB_w, dstloc=dstloc,
            invdeg=invdeg_bc, batch_local=batch_local, invcnt=invcnt,
            xT=xT_bf, peT=peT_pack, iota=iota, ident=ident, onesrow=onesrow,
            Wl=np.asarray(Wl_b), Wr_bl=Wr_bl, Wp=Wp_pack, Wi=Wi_pack,
            bi_p=bi_pack, Wo=Wo_pack, bo_p=bo_pack,
        ))

    return dims, in_maps


def build_program(dims):
    N, TAB, SPLIT = dims["N"], dims["TAB"], dims["SPLIT"]
    NB, CPA, CPB2 = dims["NB"], dims["CPA"], dims["CPB2"]
    CPB_TOT = CPA + CPB2
    NODES_PC, CH, NGRP = dims["NODES_PC"], dims["CH"], dims["NGRP"]
    KP, KI, KO = dims["KP"], dims["KI"], dims["KO"]
    B1 = SPLIT + 1

    nc = bacc.Bacc("TRN2", target_bir_lowering=False)

    t_tab = nc.dram_tensor("tab", [TAB, 128], dt.bfloat16, kind="ExternalInput")
    t_idxA = nc.dram_tensor("idxA", [128, dims["LA16"]], dt.int16, kind="ExternalInput")
    t_idxB = nc.dram_tensor("idxB", [128, dims["LB16"]], dt.int16, kind="ExternalInput")
    t_dstloc = nc.dram_tensor("dstloc", [128, CH], dt.float32, kind="ExternalInput")
    t_invdeg = nc.dram_tensor("invdeg", [64, NODES_PC], dt.float32, kind="ExternalInput")
    t_batchl = nc.dram_tensor("batch_local", [128, NB], dt.float32, kind="ExternalInput")
    t_invcnt = nc.dram_tensor("invcnt", [128, 1], dt.float32, kind="ExternalInput")
    t_xT = nc.dram_tensor("xT", [65, NODES_PC], dt.bfloat16, kind="ExternalInput")
    t_peT = nc.dram_tensor("peT", [128, KP * 128], dt.float32, kind="ExternalInput")
    t_iota = nc.dram_tensor("iota", [128, 128], dt.float32, kind="ExternalInput")
    t_ident = nc.dram_tensor("ident", [128, 128], dt.float32, kind="ExternalInput")
    t_ones = nc.dram_tensor("onesrow", [128, 128], dt.float32, kind="ExternalInput")
    t_Wl = nc.dram_tensor("Wl", [64, HID], dt.bfloat16, kind="ExternalInput")
    t_Wrbl = nc.dram_tensor("Wr_bl", [65, HID], dt.bfloat16, kind="ExternalInput")
    t_Wp = nc.dram_tensor("Wp", [128, KP * HID], dt.float32, kind="ExternalInput")
    t_Wi = nc.dram_tensor("Wi", [128, KI * HID], dt.float32, kind="ExternalInput")
    t_bi = nc.dram_tensor("bi_p", [128, HID], dt.float32, kind="ExternalInput")
    t_Wo = nc.dram_tensor("Wo", [128, KO], dt.float32, kind="ExternalInput")
    t_bo = nc.dram_tensor("bo_p", [128, 1], dt.float32, kind="ExternalInput")
    t_out = nc.dram_tensor("out", [128, 1], dt.float32, kind="ExternalOutput")

    AOT = mybir.AluOpType
    ACT = mybir.ActivationFunctionType

    with TileContext(nc) as tc:
        nc.gpsimd.load_library(library_config.mlp)
        with (
            tc.tile_pool(name="const", bufs=1) as cp,
            tc.tile_pool(name="pers_psum", bufs=1, space="PSUM") as ppool,
        ):
            # constant loads
            def load(name, tdram, shape, dtype):
                t = cp.tile(shape, dtype, tag=name)
                nc.sync.dma_start(t[:], tdram[:])
                return t

            iota_sb = load("iota", t_iota, [128, 128], dt.float32)
            ident_sb = load("ident", t_ident, [128, 128], dt.float32)
            ones_sb = load("onesrow", t_ones, [128, 128], dt.float32)
            dstloc_sb = load("dstloc", t_dstloc, [128, CH], dt.float32)
            invdeg_sb = load("invdeg", t_invdeg, [64, NODES_PC], dt.float32)
            batchl_sb = load("batch_local", t_batchl, [128, NB], dt.float32)
            invcnt_sb = load("invcnt", t_invcnt, [128, 1], dt.float32)
            xT_sb = load("xT", t_xT, [65, NODES_PC], dt.bfloat16)
            idxA_sb = load("idxA", t_idxA, [128, dims["LA16"]], dt.int16)
            idxB_sb = load("idxB", t_idxB, [128, dims["LB16"]], dt.int16)
            Wl_sb = load("Wl", t_Wl, [64, HID], dt.bfloat16)
            Wrbl_sb = load("Wr_bl", t_Wrbl, [65, HID], dt.bfloat16)
            peT_sb = load("peT", t_peT, [128, KP * 128], dt.float32)
            Wp_sb = load("Wp", t_Wp, [128, KP * HID], dt.float32)
            Wi_sb = load("Wi", t_Wi, [128, KI * HID], dt.float32)
            bi_sb = load("bi_p", t_bi, [128, HID], dt.float32)
            Wo_sb = load("Wo", t_Wo, [128, KO], dt.float32)
            bo_sb = load("bo_p", t_bo, [128, 1], dt.float32)

            ps_pool = ppool.tile([128, HID], dt.float32, tag="ps_pool")

            with (
                tc.tile_pool(name="gath", bufs=2) as gp,
                tc.tile_pool(name="work", bufs=3) as wp,
                tc.tile_pool(name="mpsum", bufs=2, space="PSUM") as mp,
            ):
                gA = gB = None
                gA_blk0 = 0
                for b in range(NB):
                    g = b // GB
                    if b % GB == 0:
                        nblk = min(NB, (g + 1) * GB) - g * GB
                        gA_blk0 = g * GB
                        nA = nblk * CPA * 128
                        cA0 = g * GB * CPA * 8
                        gA = gp.tile([128, nblk * CPA, 128], dt.bfloat16, tag="gA")
                        nc.gpsimd.dma_gather(
                            gA[:], t_tab[0:B1, :],
                            idxA_sb[:, cA0:cA0 + nA // 16], nA, nA, 128, single_packet=(nA <= 1024))
                        nB_ = nblk * CPB2 * 128
                        cB0 = g * GB * CPB2 * 8
                        gB = gp.tile([128, nblk * CPB2, 128], dt.bfloat16, tag="gB")
                        nc.gpsimd.dma_gather(
                            gB[:], t_tab[B1:TAB, :],
                            idxB_sb[:, cB0:cB0 + nB_ // 16], nB_, nB_, 128, single_packet=(nB_ <= 1024))
                    lb = b - gA_blk0

                    ps_aggr = mp.tile([64, 128], dt.float32, tag="ps_aggr")
                    for c in range(CPB_TOT):
                        oh = wp.tile([128, 128], dt.bfloat16, tag="oh")
                        nc.vector.tensor_scalar(
                            oh[:], iota_sb[:],
                            dstloc_sb[:, b * CPB_TOT + c:b * CPB_TOT + c + 1],
                            None, AOT.is_equal)
                        gsl = (gA[:, lb * CPA + c, 0:64] if c < CPA
                               else gB[:, lb * CPB2 + (c - CPA), 0:64])
                        nc.tensor.matmul(ps_aggr[:], gsl, oh[:],
                                         start=(c == 0), stop=(c == CPB_TOT - 1))

                    # aggrT (bf16) = psum * invdeg  [64, 128]
                    aggrT = wp.tile([64, 128], dt.bfloat16, tag="aggrT")
                    nc.vector.tensor_tensor(
                        aggrT[:], ps_aggr[:],
                        invdeg_sb[:, b * 128:(b + 1) * 128], AOT.mult)

                    # drug_x pre-activation [128 nodes, 256]
                    ps_drug = mp.tile([128, HID], dt.float32, tag="ps_drug")
                    nc.tensor.matmul(ps_drug[:], aggrT[:], Wl_sb[:],
                                     start=True, stop=False)
                    nc.tensor.matmul(ps_drug[:], xT_sb[:, b * 128:(b + 1) * 128],
                                     Wrbl_sb[:], start=False, stop=True)

                    H = wp.tile([128, HID], dt.bfloat16, tag="H")
                    nc.scalar.activation(H[:], ps_drug[:], ACT.Relu)

                    ohp = wp.tile([128, 128], dt.bfloat16, tag="ohp")
                    nc.vector.tensor_scalar(
                        ohp[:], iota_sb[:], batchl_sb[:, b:b + 1], None, AOT.is_equal)
                    nc.tensor.matmul(ps_pool[:], ohp[:], H[:],
                                     start=(b == 0), stop=(b == NB - 1))

            # ---------------- tail: per-core 128 graphs ----------------
            with (
                tc.tile_pool(name="tail", bufs=1) as tp,
                tc.tile_pool(name="tpsum1", bufs=1, space="PSUM") as tp1,
                tc.tile_pool(name="tpsum2", bufs=2, space="PSUM") as tp2,
            ):
                pooled = tp.tile([128, HID], dt.float32, tag="pooled")
                nc.vector.tensor_scalar(pooled[:], ps_pool[:], invcnt_sb[:, 0:1],
                                        None, AOT.mult)

                ps_prot = tp1.tile([128, HID], dt.float32, tag="ps_prot")
                for k in range(KP):
                    nc.tensor.matmul(ps_prot[:], peT_sb[:, k * 128:(k + 1) * 128],
                                     Wp_sb[:, k * HID:(k + 1) * HID],
                                     start=(k == 0), stop=(k == KP - 1))
                prot = tp.tile([128, HID], dt.float32, tag="prot")
                nc.scalar.copy(prot[:], ps_prot[:])

                # transpose combined = [pooled | prot] -> 4 lhsT tiles [128,128]
                combT = []
                for i, srctile in enumerate((pooled, prot)):
                    for j in range(HID // 128):
                        ps_t = tp2.tile([128, 128], dt.float32, tag="ps_t")
                        nc.tensor.transpose(ps_t[:], srctile[:, j * 128:(j + 1) * 128],
                                            ident_sb[:])
                        ct = tp.tile([128, 128], dt.float32, tag=f"combT{i}{j}")
                        nc.scalar.copy(ct[:], ps_t[:])
                        combT.append(ct)

                ps_int = tp1.tile([128, HID], dt.float32, tag="ps_int")
                for k in range(KI):
                    nc.tensor.matmul(ps_int[:], combT[k][:],
                                     Wi_sb[:, k * HID:(k + 1) * HID],
                                     start=(k == 0), stop=False)
                nc.tensor.matmul(ps_int[:], ones_sb[:], bi_sb[:],
                                 start=False, stop=True)
                inter = tp.tile([128, HID], dt.float32, tag="inter")
                nc.scalar.activation(inter[:], ps_int[:], ACT.Relu)

                interT = []
                for j in range(KO):
                    ps_t = tp2.tile([128, 128], dt.float32, tag="ps_t")
                    nc.tensor.transpose(ps_t[:], inter[:, j * 128:(j + 1) * 128],
                                        ident_sb[:])
                    it = tp.tile([128, 128], dt.float32, tag=f"interT{j}")
                    nc.scalar.copy(it[:], ps_t[:])
                    interT.append(it)

                ps_o = tp1.tile([128, 1], dt.float32, tag="ps_o")
                for k in range(KO):
                    nc.tensor.matmul(ps_o[:], interT[k][:], Wo_sb[:, k:k + 1],
                                     start=(k == 0), stop=False)
                nc.tensor.matmul(ps_o[:], ones_sb[:], bo_sb[:],
                                 start=False, stop=True)
                out_sb = tp.tile([128, 1], dt.float32, tag="out_sb")
                nc.scalar.copy(out_sb[:], ps_o[:])
                nc.sync.dma_start(t_out[:], out_sb[:])

    nc.compile()
    return nc


_prog_cache = {}


def _get_program(dims):
    key = tuple(sorted(dims.items()))
    if key not in _prog_cache:
        _prog_cache[key] = build_program(dims)
    return _prog_cache[key]


def kernel(**inputs):
    dims, in_maps = preprocess(**inputs)
    nc = _get_program(dims)
    res = run_bass_kernel_spmd(nc, in_maps, core_ids=list(range(dims["ncores"])))
    gpc = dims["GPC"]
    out = np.concatenate([np.asarray(r["out"])[:gpc] for r in res.results], axis=0)
    return out.astype(np.float32)
